# revision 13
# baseline (speedup 1.0000x reference)
"""Antialiased bicubic 4x downscale (blur -> bicubic/2, twice) on 8 TRN2 cores.

The whole chain is linear and separable: every stage is M_H (x) M_W acting on
the H/W axes, so the composition collapses to a single 1024->256 banded matrix
T applied on both sides: out = T @ X @ T^T per (batch, channel) image.

Sharding: pure data parallel - batch 16 -> 2 per core, 6 images/core.

Per image on-chip (v5c):
  x arrives as two half-width SWDGE DMAs (pipelined at HBM line rate).
  After half ch is in:
    pass 1 (f32r): Y quarters [ih, ch] = sum_p Tt-block.T @ X-half into
       per-quarter 1-bank PSUM tiles (banded zero-padded stationary blocks),
    evac each [128, 512] quarter (cast to bf16) as soon as it is complete,
    transpose the 8 same-ch [128,128] blocks on PE (4 per PSUM bank via
       start=False overwrite semantics).
  After ch=1: pass 2 (bf16, fast weight load): Z[ih-slice, j] =
       sum_qc Yt[qc, ih-slice].T @ Ttb[qc]; evac + store per ih-half.
Consts load once on the scalar HWDGE ring, host-prearranged to SBUF layout;
z stores ride the sync HWDGE ring; out is declared in SBUF layout
[128, 6, 2, 256] and unscrambled on the host.
"""

import numpy as np
import ml_dtypes

import concourse.bacc as bacc
import concourse.mybir as mybir
import concourse.tile as tile
from concourse.bass_utils import run_bass_kernel_spmd

SIGMA = 0.66
BICUBIC_W = np.array([-0.09375, 0.59375, 0.59375, -0.09375], dtype=np.float64)

N_CORES = 8
B, C, H, W = 16, 3, 1024, 1024
HO = H // 4
IMGS = (B // N_CORES) * C  # 6 images per core

F32 = mybir.dt.float32
F32R = mybir.dt.float32r
BF16 = mybir.dt.bfloat16


def _gauss_matrix(n):
    x = np.arange(3, dtype=np.float32) - np.float32(1.0)
    k = np.exp(np.float32(-0.5) * (x / np.float32(SIGMA)) ** 2)
    k = (k / k.sum()).astype(np.float64)
    G = np.zeros((n, n))
    for t in range(3):
        G += k[t] * np.eye(n, n, t - 1)
    return G


def _down_matrix(n):
    # out[i] = sum_t w[t] * x[clamp(2i + t - 1, 0, n-1)]
    m = n // 2
    D = np.zeros((m, n))
    for i in range(m):
        for t in range(4):
            j = min(max(2 * i + t - 1, 0), n - 1)
            D[i, j] += BICUBIC_W[t]
    return D


def build_T():
    T = _down_matrix(H // 2) @ _gauss_matrix(H // 2) @ _down_matrix(H) @ _gauss_matrix(H)
    return T.astype(np.float32)  # [256, 1024]


def _pass1_pieces(Tt):
    """(pc, ih) pairs where Tt[128pc:128pc+128, 128ih:128ih+128] is nonzero."""
    pieces = []
    for ih in range(2):
        for pc in range(8):
            if np.any(Tt[128 * pc : 128 * (pc + 1), 128 * ih : 128 * (ih + 1)]):
                pieces.append((pc, ih))
    return pieces


def _build_graph():
    Tt = build_T().T  # [1024, 256]
    pieces = _pass1_pieces(Tt)

    nc = bacc.Bacc("TRN2", target_bir_lowering=False, debug=False)
    x = nc.dram_tensor("x", [IMGS, H, W], F32R, kind="ExternalInput").ap()
    # tb is host-prearranged to the SBUF layout: tb[p, c, n] = Tt[128c+p, n]
    tb = nc.dram_tensor("tb", [128, 8, HO], BF16, kind="ExternalInput").ap()
    eye = nc.dram_tensor("eye", [128, 128], BF16, kind="ExternalInput").ap()
    # out in SBUF layout [p, img, c, j] = Z[img, 128c+p, j]; host unscrambles
    out = nc.dram_tensor("out", [128, IMGS, 2, HO], F32, kind="ExternalOutput").ap()

    with tile.TileContext(nc) as tc:
        with (
            tc.tile_pool(name="const", bufs=1) as cpool,
            tc.tile_pool(name="xin", bufs=6) as xpool,
            tc.tile_pool(name="ysb", bufs=2) as ypool,
            tc.tile_pool(name="ytsb", bufs=2) as ytpool,
            tc.tile_pool(name="zout", bufs=2) as zpool,
            tc.tile_pool(name="psy", bufs=4, space="PSUM") as psy,
            tc.tile_pool(name="pst", bufs=2, space="PSUM") as pst,
            tc.tile_pool(name="ps2", bufs=2, space="PSUM") as ps2,
        ):
            ttb = cpool.tile([128, 8, HO], BF16, tag="ttb")
            nc.scalar.dma_start(out=ttb[:], in_=tb)
            ident = cpool.tile([128, 128], BF16, tag="ident")
            nc.scalar.dma_start(out=ident[:], in_=eye)
            # f32r copy of Tt for pass 1 stationary, cast on-chip
            tt = cpool.tile([128, 8, HO], F32R, tag="tt")
            nc.vector.tensor_copy(tt[:], ttb[:])

            for img in range(IMGS):
                xt = [
                    xpool.tile([128, 8, W // 2], F32R, tag="xt", name=f"xt{img}_{ch}")
                    for ch in range(2)
                ]
                xr = x[img].rearrange("(c p) (ch w) -> p ch c w", p=128, ch=2)
                for ch in range(2):
                    nc.gpsimd.dma_start(out=xt[ch][:], in_=xr[:, ch])

                y_sb = ypool.tile([128, 2, W], BF16)
                yt_sb = ytpool.tile([128, 8, HO], BF16)
                if img % 2 == 0:
                    zpair = zpool.tile([128, 2, 2, HO], F32, tag="zout",
                                       name=f"z{img}")
                z = zpair[:, img % 2]

                for ch in range(2):
                    for ih in range(2):
                        # pass 1 quarter (ih, ch) into a 1-bank PSUM tile
                        yq = psy.tile(
                            [128, 512], F32, tag="psy", name=f"psy{img}_{ch}_{ih}"
                        )
                        pcs = [pc for (pc, ih2) in pieces if ih2 == ih]
                        for k, pc in enumerate(pcs):
                            nc.tensor.matmul(
                                yq[:],
                                tt[:, pc, 128 * ih : 128 * (ih + 1)],
                                xt[ch][:, pc, :],
                                start=(k == 0),
                                stop=(k == len(pcs) - 1),
                            )
                        dst = y_sb[:, ih, 512 * ch : 512 * (ch + 1)]
                        if ih == 0:
                            nc.vector.tensor_copy(dst, yq[:])
                        else:
                            nc.scalar.copy(dst, yq[:])
                    # transpose the 8 same-ch blocks: qc in [4ch, 4ch+4)
                    for ih in range(2):
                        tp = pst.tile(
                            [128, 512], BF16, tag="pst", name=f"tp{img}_{ch}_{ih}"
                        )
                        for s in range(4):
                            qc = 4 * ch + s
                            nc.tensor.matmul(
                                tp[:, 128 * s : 128 * (s + 1)],
                                y_sb[:, ih, 128 * qc : 128 * (qc + 1)],
                                ident[:],
                                is_transpose=True,
                                start=(s == 0),
                                stop=(s == 3),
                            )
                        dst = yt_sb[
                            :, 4 * ch : 4 * ch + 4, 128 * ih : 128 * (ih + 1)
                        ]
                        src = tp[:].rearrange("p (s w) -> p s w", s=4)
                        if ih == 0:
                            nc.vector.tensor_copy(dst, src)
                        else:
                            nc.scalar.copy(dst, src)

                # pass 2: Z[ih-slice, j] = sum_qc Yt[qc, ih-slice].T @ Ttb[qc]
                for ih in range(2):
                    acc = ps2.tile([128, HO], F32, tag="ps2", name=f"ps2_{img}_{ih}")
                    for qc in range(8):
                        nc.tensor.matmul(
                            acc[:],
                            yt_sb[:, qc, 128 * ih : 128 * (ih + 1)],
                            ttb[:, qc, :],
                            start=(qc == 0),
                            stop=(qc == 7),
                        )
                    if ih == 0:
                        nc.vector.tensor_copy(z[:, ih, :], acc[:])
                    else:
                        nc.scalar.copy(z[:, ih, :], acc[:])
                if img % 2 == 1:
                    nc.gpsimd.dma_start(
                        out=out[:, img - 1 : img + 1], in_=zpair[:]
                    )
    nc.compile()
    return nc


_GRAPH = None


def _get_graph():
    global _GRAPH
    if _GRAPH is None:
        _GRAPH = _build_graph()
    return _GRAPH


def run(x, **spmd_kwargs):
    x = np.ascontiguousarray(np.asarray(x, dtype=np.float32))
    assert x.shape == (B, C, H, W)
    nc = _get_graph()
    Tt = build_T().T  # [1024, 256] f32
    tb_host = np.ascontiguousarray(
        Tt.reshape(8, 128, HO).transpose(1, 0, 2)
    ).astype(ml_dtypes.bfloat16)
    eye_host = np.eye(128, dtype=ml_dtypes.bfloat16)
    per_core = B // N_CORES
    in_maps = [
        {
            "x": x[i * per_core : (i + 1) * per_core].reshape(IMGS, H, W),
            "tb": tb_host,
            "eye": eye_host,
        }
        for i in range(N_CORES)
    ]
    res = run_bass_kernel_spmd(nc, in_maps, core_ids=list(range(N_CORES)), **spmd_kwargs)
    outs = []
    for r in res.results:
        o = r["out"].transpose(1, 2, 0, 3).reshape(IMGS, 2 * 128, HO)
        outs.append(o.reshape(per_core, C, HO, HO))
    return np.concatenate(outs, axis=0), res


def kernel(x):
    out, _ = run(x)
    return out


# revision 14
# speedup vs baseline: 1.1359x; 1.1359x over previous
"""Antialiased bicubic 4x downscale (blur -> bicubic/2, twice) on 8 TRN2 cores.

The whole chain is linear and separable: every stage is M_H (x) M_W acting on
the H/W axes, so the composition collapses to a single 1024->256 banded matrix
T applied on both sides: out = T @ X @ T^T per (batch, channel) image.

Sharding: pure data parallel - batch 16 -> 2 per core, 6 images/core.

Per image on-chip (v5c):
  x arrives as two half-width SWDGE DMAs (pipelined at HBM line rate).
  After half ch is in:
    pass 1 (f32r): Y quarters [ih, ch] = sum_p Tt-block.T @ X-half into
       per-quarter 1-bank PSUM tiles (banded zero-padded stationary blocks),
    evac each [128, 512] quarter (cast to bf16) as soon as it is complete,
    transpose the 8 same-ch [128,128] blocks on PE (4 per PSUM bank via
       start=False overwrite semantics).
  After ch=1: pass 2 (bf16, fast weight load): Z[ih-slice, j] =
       sum_qc Yt[qc, ih-slice].T @ Ttb[qc]; evac + store per ih-half.
Consts load once on the scalar HWDGE ring, host-prearranged to SBUF layout;
z stores ride the sync HWDGE ring; out is declared in SBUF layout
[128, 6, 2, 256] and unscrambled on the host.
"""

import numpy as np
import ml_dtypes

import concourse.bacc as bacc
import concourse.mybir as mybir
import concourse.tile as tile
from concourse.bass_utils import run_bass_kernel_spmd

SIGMA = 0.66
BICUBIC_W = np.array([-0.09375, 0.59375, 0.59375, -0.09375], dtype=np.float64)

N_CORES = 8
B, C, H, W = 16, 3, 1024, 1024
HO = H // 4
IMGS = (B // N_CORES) * C  # 6 images per core

F32 = mybir.dt.float32
F32R = mybir.dt.float32r
BF16 = mybir.dt.bfloat16


def _gauss_matrix(n):
    x = np.arange(3, dtype=np.float32) - np.float32(1.0)
    k = np.exp(np.float32(-0.5) * (x / np.float32(SIGMA)) ** 2)
    k = (k / k.sum()).astype(np.float64)
    G = np.zeros((n, n))
    for t in range(3):
        G += k[t] * np.eye(n, n, t - 1)
    return G


def _down_matrix(n):
    # out[i] = sum_t w[t] * x[clamp(2i + t - 1, 0, n-1)]
    m = n // 2
    D = np.zeros((m, n))
    for i in range(m):
        for t in range(4):
            j = min(max(2 * i + t - 1, 0), n - 1)
            D[i, j] += BICUBIC_W[t]
    return D


def build_T():
    T = _down_matrix(H // 2) @ _gauss_matrix(H // 2) @ _down_matrix(H) @ _gauss_matrix(H)
    return T.astype(np.float32)  # [256, 1024]


def _pass1_pieces(Tt):
    """(pc, ih) pairs where Tt[128pc:128pc+128, 128ih:128ih+128] is nonzero."""
    pieces = []
    for ih in range(2):
        for pc in range(8):
            if np.any(Tt[128 * pc : 128 * (pc + 1), 128 * ih : 128 * (ih + 1)]):
                pieces.append((pc, ih))
    return pieces


def _build_graph():
    Tt = build_T().T  # [1024, 256]
    pieces = _pass1_pieces(Tt)

    nc = bacc.Bacc("TRN2", target_bir_lowering=False, debug=False)
    x = nc.dram_tensor("x", [IMGS, H, W], F32R, kind="ExternalInput").ap()
    # tb is host-prearranged to the SBUF layout: tb[p, c, n] = Tt[128c+p, n]
    tb = nc.dram_tensor("tb", [128, 8, HO], BF16, kind="ExternalInput").ap()
    eye = nc.dram_tensor("eye", [128, 128], BF16, kind="ExternalInput").ap()
    # out in SBUF layout [p, img, c, j] = Z[img, 128c+p, j]; host unscrambles
    out = nc.dram_tensor("out", [128, IMGS, 2, HO], F32, kind="ExternalOutput").ap()

    with tile.TileContext(nc) as tc:
        with (
            tc.tile_pool(name="const", bufs=1) as cpool,
            tc.tile_pool(name="xin", bufs=6) as xpool,
            tc.tile_pool(name="ysb", bufs=2) as ypool,
            tc.tile_pool(name="ytsb", bufs=2) as ytpool,
            tc.tile_pool(name="zout", bufs=2) as zpool,
            tc.tile_pool(name="psy", bufs=4, space="PSUM") as psy,
            tc.tile_pool(name="pst", bufs=2, space="PSUM") as pst,
            tc.tile_pool(name="ps2", bufs=2, space="PSUM") as ps2,
        ):
            ttb = cpool.tile([128, 8, HO], BF16, tag="ttb")
            nc.scalar.dma_start(out=ttb[:], in_=tb)
            ident = cpool.tile([128, 128], BF16, tag="ident")
            nc.scalar.dma_start(out=ident[:], in_=eye)
            # f32r copy of Tt for pass 1 stationary, cast on-chip
            tt = cpool.tile([128, 8, HO], F32R, tag="tt")
            nc.vector.tensor_copy(tt[:], ttb[:])

            def transposes(img, y_sb, yt_sb, ih, qcs, name):
                """PE-transpose blocks (qc in qcs, fixed ih) of y_sb into
                yt_sb, 4 per PSUM bank; qcs must be 4 consecutive blocks."""
                tp = pst.tile([128, 512], BF16, tag="pst", name=name)
                for s, qc in enumerate(qcs):
                    nc.tensor.matmul(
                        tp[:, 128 * s : 128 * (s + 1)],
                        y_sb[:, ih, 128 * qc : 128 * (qc + 1)],
                        ident[:],
                        is_transpose=True,
                        start=(s == 0),
                        stop=(s == len(qcs) - 1),
                    )
                dst = yt_sb[:, qcs[0] : qcs[0] + 4, 128 * ih : 128 * (ih + 1)]
                src = tp[:].rearrange("p (s w) -> p s w", s=4)
                if ih == 0:
                    nc.vector.tensor_copy(dst, src)
                else:
                    nc.scalar.copy(dst, src)

            def pass2(img, yt_sb, z, ih):
                acc = ps2.tile([128, HO], F32, tag="ps2", name=f"ps2_{img}_{ih}")
                for qc in range(8):
                    nc.tensor.matmul(
                        acc[:],
                        yt_sb[:, qc, 128 * ih : 128 * (ih + 1)],
                        ttb[:, qc, :],
                        start=(qc == 0),
                        stop=(qc == 7),
                    )
                if ih == 0:
                    nc.vector.tensor_copy(z[:, ih, :], acc[:])
                else:
                    nc.scalar.copy(z[:, ih, :], acc[:])

            for img in range(IMGS):
                y_sb = ypool.tile([128, 2, W], BF16)
                yt_sb = ytpool.tile([128, 8, HO], BF16)
                if img % 2 == 0:
                    zpair = zpool.tile([128, 2, 2, HO], F32, tag="zout",
                                       name=f"z{img}")
                z = zpair[:, img % 2]

                if img < IMGS - 1:
                    # column-halved loads; per-half pass1 + transposes
                    xt = [
                        xpool.tile([128, 8, W // 2], F32R, tag="xt",
                                   name=f"xt{img}_{ch}")
                        for ch in range(2)
                    ]
                    xr = x[img].rearrange(
                        "(c p) (ch w) -> p ch c w", p=128, ch=2
                    )
                    for ch in range(2):
                        nc.gpsimd.dma_start(out=xt[ch][:], in_=xr[:, ch])

                    for ch in range(2):
                        for ih in range(2):
                            yq = psy.tile(
                                [128, 512], F32, tag="psy",
                                name=f"psy{img}_{ch}_{ih}",
                            )
                            pcs = [pc for (pc, ih2) in pieces if ih2 == ih]
                            for k, pc in enumerate(pcs):
                                nc.tensor.matmul(
                                    yq[:],
                                    tt[:, pc, 128 * ih : 128 * (ih + 1)],
                                    xt[ch][:, pc, :],
                                    start=(k == 0),
                                    stop=(k == len(pcs) - 1),
                                )
                            dst = y_sb[:, ih, 512 * ch : 512 * (ch + 1)]
                            if ih == 0:
                                nc.vector.tensor_copy(dst, yq[:])
                            else:
                                nc.scalar.copy(dst, yq[:])
                        for ih in range(2):
                            transposes(img, y_sb, yt_sb, ih,
                                       list(range(4 * ch, 4 * ch + 4)),
                                       f"tp{img}_{ch}_{ih}")
                    for ih in range(2):
                        pass2(img, yt_sb, z, ih)
                else:
                    # last image: row-split (pcs 0-4, then 5-7) so only the
                    # ih=1 half of the pipeline depends on the final bytes
                    xa = xpool.tile([128, 5, W], F32R, tag="xt",
                                    name=f"xa{img}")
                    xb = xpool.tile([128, 3, W], F32R, tag="xt",
                                    name=f"xb{img}")
                    xr = x[img].rearrange("(c p) w -> p c w", p=128)
                    nc.gpsimd.dma_start(out=xa[:], in_=xr[:, 0:5])
                    nc.gpsimd.dma_start(out=xb[:], in_=xr[:, 5:8])

                    yqs = {}
                    for ih in range(2):
                        for ch in range(2):
                            yqs[ih, ch] = psy.tile(
                                [128, 512], F32, tag="psy",
                                name=f"psy{img}_{ch}_{ih}",
                            )
                    pcs0 = [pc for (pc, ih2) in pieces if ih2 == 0]
                    pcs1 = [pc for (pc, ih2) in pieces if ih2 == 1]
                    # chunk A (pcs 0-4): all of ih0, and ih1's pcs {3, 4}
                    for ch in range(2):
                        for k, pc in enumerate(pcs0):
                            nc.tensor.matmul(
                                yqs[0, ch][:],
                                tt[:, pc, 0:128],
                                xa[:, pc, 512 * ch : 512 * (ch + 1)],
                                start=(k == 0),
                                stop=(k == len(pcs0) - 1),
                            )
                        for k, pc in enumerate([p for p in pcs1 if p < 5]):
                            nc.tensor.matmul(
                                yqs[1, ch][:],
                                tt[:, pc, 128:256],
                                xa[:, pc, 512 * ch : 512 * (ch + 1)],
                                start=(k == 0),
                                stop=False,
                            )
                    # ih0 half fully finishes during chunk B's DMA
                    for ch in range(2):
                        nc.vector.tensor_copy(
                            y_sb[:, 0, 512 * ch : 512 * (ch + 1)],
                            yqs[0, ch][:],
                        )
                    for qq in range(2):
                        transposes(img, y_sb, yt_sb, 0,
                                   list(range(4 * qq, 4 * qq + 4)),
                                   f"tp{img}_{qq}_0")
                    pass2(img, yt_sb, z, 0)
                    # chunk B (pcs 5-7) completes ih1
                    for ch in range(2):
                        rest = [p for p in pcs1 if p >= 5]
                        for k, pc in enumerate(rest):
                            nc.tensor.matmul(
                                yqs[1, ch][:],
                                tt[:, pc, 128:256],
                                xb[:, pc - 5, 512 * ch : 512 * (ch + 1)],
                                start=False,
                                stop=(k == len(rest) - 1),
                            )
                        nc.scalar.copy(
                            y_sb[:, 1, 512 * ch : 512 * (ch + 1)],
                            yqs[1, ch][:],
                        )
                    for qq in range(2):
                        transposes(img, y_sb, yt_sb, 1,
                                   list(range(4 * qq, 4 * qq + 4)),
                                   f"tp{img}_{qq}_1")
                    pass2(img, yt_sb, z, 1)
                if img % 2 == 1:
                    nc.gpsimd.dma_start(
                        out=out[:, img - 1 : img + 1], in_=zpair[:]
                    )
    nc.compile()
    return nc


_GRAPH = None


def _get_graph():
    global _GRAPH
    if _GRAPH is None:
        _GRAPH = _build_graph()
    return _GRAPH


def run(x, **spmd_kwargs):
    x = np.ascontiguousarray(np.asarray(x, dtype=np.float32))
    assert x.shape == (B, C, H, W)
    nc = _get_graph()
    Tt = build_T().T  # [1024, 256] f32
    tb_host = np.ascontiguousarray(
        Tt.reshape(8, 128, HO).transpose(1, 0, 2)
    ).astype(ml_dtypes.bfloat16)
    eye_host = np.eye(128, dtype=ml_dtypes.bfloat16)
    per_core = B // N_CORES
    in_maps = [
        {
            "x": x[i * per_core : (i + 1) * per_core].reshape(IMGS, H, W),
            "tb": tb_host,
            "eye": eye_host,
        }
        for i in range(N_CORES)
    ]
    res = run_bass_kernel_spmd(nc, in_maps, core_ids=list(range(N_CORES)), **spmd_kwargs)
    outs = []
    for r in res.results:
        o = r["out"].transpose(1, 2, 0, 3).reshape(IMGS, 2 * 128, HO)
        outs.append(o.reshape(per_core, C, HO, HO))
    return np.concatenate(outs, axis=0), res


def kernel(x):
    out, _ = run(x)
    return out
